# revision 25
# baseline (speedup 1.0000x reference)
"""Banded causal self-attention (B=1, T=4096, C=1024, H=16, Dh=64, band=128)
on 8 Trainium2 NeuronCores, sequence-parallel (512 queries/core + 128-row halo).

v2: software-pipelined issue order.  The v1 kernel ran all projections, then
all attention, then the output projection; the attention phase's small
matmuls (128-256 free cols, fresh LDWEIGHTS each) left the PE duty cycle low
enough that the HAM clock gate re-throttled to K=4/8 (1.2 GHz) for ~65us.
v2 interleaves the second half of the V projection (dense 512-col matmuls)
into the attention instruction stream at matmul granularity, keeping the PE
busy window saturated so HAM stays at 8/8, and moves P^T copies to the idle
GPSIMD engine, DMA issue off the hot sync queue, and q/k tiles to bf16
(FWL-eligible stationary operands).

Layout strategy (host pre-transposes, so zero on-chip weight transposes):
  - feeds x^T slice (C, 640) per core; Wq^T (scaled by 1/sqrt(Dh)), Wk^T,
    Wv^T, Wo^T as (C, C) contraction-major arrays.
  - q^T/k^T computed as (o, t) tiles; v as (t, o); attention scores banded
    (each 128-query block sees exactly 2 key blocks); softmax along free dim
    without max-subtraction (scores are O(1) by construction); P transposed
    via PE; y^T accumulated per head; output projection back to (t, u).
"""

import os
import sys
from collections import deque

import ml_dtypes
import numpy as np

sys.path.insert(0, "/opt/trn_rl_repo")

import concourse.bass as bass  # noqa: F401
import concourse.mybir as mybir
import concourse.tile as tile
from concourse import bacc
from concourse.bass_utils import run_bass_kernel_spmd
from concourse.masks import make_identity

T, C, H, DH = 4096, 1024, 16, 64
BAND = 128
NCORES = 8
TLOC = T // NCORES          # 512 queries per core
HALO = BAND                 # 128
KV = TLOC + HALO            # 640 kv rows per core
NQB = TLOC // 128           # 4 query blocks
NKB = KV // 128             # 5 kv blocks
KT = C // 128               # 8 contraction tiles
F32 = mybir.dt.float32
BF16 = mybir.dt.bfloat16
MULT = mybir.AluOpType.mult
ADD = mybir.AluOpType.add
EXP = mybir.ActivationFunctionType.Exp

_cached = {}


def build_nc():
    nc = bacc.Bacc(
        "TRN2",
        target_bir_lowering=False,
        debug=False,
        num_devices=NCORES,
    )

    xt_d = nc.dram_tensor("xt", [C, KV], BF16, kind="ExternalInput")
    wqt_d = nc.dram_tensor("wqt", [C, C], BF16, kind="ExternalInput")
    wkt_d = nc.dram_tensor("wkt", [C, C], BF16, kind="ExternalInput")
    wvt_d = nc.dram_tensor("wvt", [C, C], BF16, kind="ExternalInput")
    wot_d = nc.dram_tensor("wot", [C, C], BF16, kind="ExternalInput")
    m0_d = nc.dram_tensor("mask0", [128, 256], BF16, kind="ExternalInput")
    mr_d = nc.dram_tensor("maskr", [128, 256], BF16, kind="ExternalInput")
    out_d = nc.dram_tensor("out", [TLOC, C], F32, kind="ExternalOutput")

    with tile.TileContext(nc) as tc:
        with (
            tc.tile_pool(name="const", bufs=1) as constp,
            tc.tile_pool(name="xt", bufs=KT) as xtp,
            tc.tile_pool(name="w", bufs=32) as wp,
            tc.tile_pool(name="qt", bufs=KT) as qtp,
            tc.tile_pool(name="kt", bufs=KT) as ktp,
            tc.tile_pool(name="v", bufs=NKB) as vp,
            tc.tile_pool(name="yt", bufs=KT) as ytp,
            tc.tile_pool(name="att", bufs=12) as attp,
            tc.tile_pool(name="pt", bufs=4) as ptp,
            tc.tile_pool(name="stat", bufs=8) as statp,
            tc.tile_pool(name="z", bufs=2) as zp,
            tc.tile_pool(name="psum", bufs=1, space="PSUM") as psp,
        ):
            # constants
            ident = constp.tile([128, 128], BF16, name="ident")
            make_identity(nc, ident[:])
            # HAM warm-up: junk matmuls that run while the first DMAs land,
            # flipping the PE clock gate to 8/8 before real work begins
            junk = constp.tile([128, 512], BF16, name="junk")
            nc.vector.memset(junk[:], 0.0)
            ps_w = psp.tile([128, 512], F32, tag="proj", bufs=2, name="warm")
            for _ in range(10):
                nc.tensor.matmul(ps_w[:], junk[:, 0:128], junk[:], start=True,
                                 stop=True)

            # ---- DMA staging, in need-order.  Issue serialization (~0.65us
            # per dma_start) staggers the streams so the first-needed tiles
            # get the HBM bandwidth first: sync issues x^T then wv/wo/masks;
            # the scalar queue issues wq/wk in parallel.
            xt_t = []
            for a in range(KT):
                xt = xtp.tile([128, KV], BF16, name=f"xt{a}", tag="xt", bufs=KT)
                nc.sync.dma_start(xt[:], xt_d[a * 128:(a + 1) * 128, :])
                xt_t.append(xt)

            def load_w(dram, base, k, eng):
                w = wp.tile([128, C], BF16, name=f"{base}{k}", tag="w", bufs=32)
                eng.dma_start(w[:], dram[k * 128:(k + 1) * 128, :])
                return w

            wq_t = [load_w(wqt_d, "wq", k, nc.scalar) for k in range(KT)]
            wk_t = [load_w(wkt_d, "wk", k, nc.scalar) for k in range(KT)]
            wv_t = [load_w(wvt_d, "wv", k, nc.sync) for k in range(KT)]
            wo_t = [load_w(wot_d, "wo", k, nc.sync) for k in range(KT)]

            mb01 = constp.tile([128, 2, 256], BF16, name="mb01")
            mbr2 = constp.tile([128, 2, 256], BF16, name="mbr2")
            nc.sync.dma_start(mb01[:, 0, :], m0_d[:])
            nc.sync.dma_start(mb01[:, 1, :], mr_d[:])
            nc.sync.dma_start(mbr2[:, 0, :], mr_d[:])
            nc.sync.dma_start(mbr2[:, 1, :], mr_d[:])

            qt_t = [None] * KT
            kt_t = [None] * KT
            v_t = []

            # Breadth-first (k-outer) projection passes for Q and K, o=0..6:
            # seven output blocks accumulate in seven PSUM banks at once, so
            # the first matmuls issue as soon as (x^T tile k, W tile k) land
            # instead of waiting for the whole weight matrix.  The attention
            # PSUM tags (proj/s/t/y) are idle this early, so their statically
            # reserved banks are borrowed.  The last two k-steps run per-o so
            # each o closes (and copies out) in ascending order.
            QTAGS = ["proj", "proj", "s", "s", "t", "t", "y"]
            NO7 = KT - 1

            def kouter_pass(mk_mm, mk_fin, tag_name):
                ps = [psp.tile([128, 512], F32, tag=QTAGS[o], bufs=2,
                               name=f"{tag_name}{o}") for o in range(NO7)]
                for k in range(KT - 2):
                    for o in range(NO7):
                        mk_mm(ps[o], o, k, k == 0, False)
                for o in range(NO7):
                    mk_mm(ps[o], o, KT - 2, False, False)
                    mk_mm(ps[o], o, KT - 1, False, True)
                    mk_fin(ps[o], o)

            def q_mm(ps, o, k, st, sp):
                nc.tensor.matmul(
                    ps[:],
                    wq_t[k][:, o * 128:(o + 1) * 128],
                    xt_t[k][:, HALO:],
                    start=st,
                    stop=sp,
                )

            def q_fin(ps, o):
                qt = qtp.tile([128, TLOC], BF16, name=f"qt{o}", tag="qt",
                              bufs=KT)
                nc.scalar.copy(qt[:], ps[:])
                qt_t[o] = qt

            def k_mm(n0, nw):
                def f(ps, o, k, st, sp):
                    nc.tensor.matmul(
                        ps[:, :nw],
                        wk_t[k][:, o * 128:(o + 1) * 128],
                        xt_t[k][:, n0:n0 + nw],
                        start=st,
                        stop=sp,
                    )
                return f

            def k_fin(n0, nw):
                def f(ps, o):
                    if kt_t[o] is None:
                        kt_t[o] = ktp.tile([128, KV], BF16, name=f"kt{o}",
                                           tag="kt", bufs=KT)
                    nc.scalar.copy(kt_t[o][:, n0:n0 + nw], ps[:, :nw])
                return f

            kouter_pass(q_mm, q_fin, "psq")
            kouter_pass(k_mm(0, 384), k_fin(0, 384), "pska")
            kouter_pass(k_mm(384, 256), k_fin(384, 256), "pskb")

            # ---- v projection, first half (output cols 0:512 = heads 0-7):
            # computed up front so attention pairs g=0..3 can run; the second
            # half is emitted lazily through the dense-matmul feeder below,
            # interleaved into the attention phase to keep the PE saturated.
            for tb in range(NKB):
                v = vp.tile([128, C], BF16, name=f"v{tb}", tag="v", bufs=NKB)
                ps = psp.tile([128, 512], F32, tag="proj", bufs=2, name=f"psv{tb}_0")
                for k in range(KT):
                    nc.tensor.matmul(
                        ps[:],
                        xt_t[k][:, tb * 128:(tb + 1) * 128],
                        wv_t[k][:, 0:512],
                        start=(k == 0),
                        stop=(k == KT - 1),
                    )
                nc.scalar.copy(v[:, 0:512], ps[:])
                v_t.append(v)

            # ---- dense-matmul feeder: one closure per PE instruction of
            # deferred dense work (V second half, K o=6,7, Q o=7).  Attention
            # code pulls from this between its own small matmuls so the PE
            # never sees a low-duty window (which would re-throttle HAM), and
            # LDWEIGHTS of the small matmuls hides under the 512-col ones.
            steps = []
            marks = {}

            def _mk_v_chain(tb):
                state = {}

                def step(k):
                    def f():
                        if k == 0:
                            state["ps"] = psp.tile(
                                [128, 512], F32, tag="proj", bufs=2,
                                name=f"psv{tb}_512")
                        nc.tensor.matmul(
                            state["ps"][:],
                            xt_t[k][:, tb * 128:(tb + 1) * 128],
                            wv_t[k][:, 512:1024],
                            start=(k == 0),
                            stop=(k == KT - 1),
                        )
                        if k == KT - 1:
                            nc.scalar.copy(v_t[tb][:, 512:1024], state["ps"][:])
                    return f

                return [step(k) for k in range(KT)]

            def _mk_k_chunk(o, n0, nw):
                state = {}

                def step(k):
                    def f():
                        if kt_t[o] is None:
                            kt_t[o] = ktp.tile([128, KV], BF16, name=f"kt{o}",
                                               tag="kt", bufs=KT)
                        if k == 0:
                            state["ps"] = psp.tile(
                                [128, 512], F32, tag="proj", bufs=2,
                                name=f"psk{o}_{n0}")
                        nc.tensor.matmul(
                            state["ps"][:, :nw],
                            wk_t[k][:, o * 128:(o + 1) * 128],
                            xt_t[k][:, n0:n0 + nw],
                            start=(k == 0),
                            stop=(k == KT - 1),
                        )
                        if k == KT - 1:
                            nc.scalar.copy(kt_t[o][:, n0:n0 + nw],
                                           state["ps"][:, :nw])
                    return f

                return [step(k) for k in range(KT)]

            def _mk_q_chain(o):
                state = {}

                def step(k):
                    def f():
                        if k == 0:
                            state["ps"] = psp.tile(
                                [128, 512], F32, tag="proj", bufs=2,
                                name=f"psq{o}")
                        nc.tensor.matmul(
                            state["ps"][:],
                            wq_t[k][:, o * 128:(o + 1) * 128],
                            xt_t[k][:, HALO:],
                            start=(k == 0),
                            stop=(k == KT - 1),
                        )
                        if k == KT - 1:
                            qt = qtp.tile([128, TLOC], BF16, name=f"qt{o}",
                                          tag="qt", bufs=KT)
                            nc.scalar.copy(qt[:], state["ps"][:])
                            qt_t[o] = qt
                    return f

                return [step(k) for k in range(KT)]

            # O-projection chains for tb=0 (both 512-col halves): the o=0..5
            # matmuls can run during the attention tail (their yt inputs are
            # ready), keeping the PE dense there; o=6,7 + the copy/DMA are
            # emitted after the last PV.
            zt0 = [None]

            def _mk_o_chain(tb, n0):
                state = {}

                def step(o):
                    def f():
                        if o == 0:
                            state["ps"] = psp.tile(
                                [128, 512], F32, tag="proj", bufs=2,
                                name=f"psz{tb}_{n0}")
                        nc.tensor.matmul(
                            state["ps"][:],
                            yt_t[o][:, tb * 128:(tb + 1) * 128],
                            wo_t[o][:, n0:n0 + 512],
                            start=(o == 0),
                            stop=(o == KT - 1),
                        )
                        if o == KT - 1:
                            if zt0[0] is None:
                                zt0[0] = zp.tile([128, C], F32, name=f"z{tb}",
                                                 tag="z", bufs=2)
                            nc.scalar.copy(zt0[0][:, n0:n0 + 512],
                                           state["ps"][:])
                            nc.gpsimd.dma_start(
                                out_d[tb * 128:(tb + 1) * 128, n0:n0 + 512],
                                zt0[0][:, n0:n0 + 512])
                    return f

                return [step(o) for o in range(KT)]

            ochain = {n0: _mk_o_chain(0, n0) for n0 in (0, 512)}

            for tb in range(NKB):
                steps.extend(_mk_v_chain(tb))
            marks["v"] = len(steps)
            for n0, nw in ((0, 384), (384, 256)):
                steps.extend(_mk_k_chunk(KT - 1, n0, nw))
            steps.extend(_mk_q_chain(KT - 1))
            marks["k7q7"] = len(steps)
            for o in range(6):
                steps.append(ochain[0][o])
                steps.append(ochain[512][o])
            marks["opre"] = len(steps)

            # feed() pops at most one step per `every` calls, and never past
            # pos["limit"]: a segment only opens once the drain before it has
            # run (its data dependencies are then guaranteed emitted).
            pos = {"i": 0, "tick": 0, "every": 3, "limit": marks["v"]}

            def feed():
                pos["tick"] += 1
                if pos["tick"] % pos["every"] == 0 and pos["i"] < pos["limit"]:
                    steps[pos["i"]]()
                    pos["i"] += 1

            def drain(mark, every=None, open_to=None):
                end = marks[mark] if mark in marks else len(steps)
                while pos["i"] < end:
                    steps[pos["i"]]()
                    pos["i"] += 1
                pos["limit"] = marks[open_to] if open_to else len(steps)
                if every is not None:
                    pos["every"] = every

            # ---- banded attention: head pairs (concurrent PE row-groups)
            # x query-block pairs batched into shared PSUM banks.
            # Pipelined: scores/softmax/transpose for pair g overlap the PV
            # matmuls of pair g-1, with dense V-projection matmuls fed
            # between attention matmuls.
            yt_t = [None] * KT
            pt_all = [None] * KT

            def emit_scores(g):
                hs = (2 * g, 2 * g + 1)
                pt_t = {h: ptp.tile([128, 256 * NKB], BF16,
                                    name=f"pt{h}", tag="pt", bufs=4)
                        for h in hs}
                pt_all[g] = pt_t
                for qp in range(NQB // 2):
                    qbs = (2 * qp, 2 * qp + 1)
                    mb = mb01 if qp == 0 else mbr2
                    ps_s, e, den, rec = {}, {}, {}, {}
                    for h in hs:
                        ho = (h % 2) * 64
                        ps_s[h] = psp.tile([128, 512], F32, tag="s", bufs=2,
                                           name=f"s{h}_{qp}")
                        for i, qb in enumerate(qbs):
                            nc.tensor.matmul(
                                ps_s[h][:, i * 256:(i + 1) * 256],
                                qt_t[g][ho:ho + 64, qb * 128:(qb + 1) * 128],
                                kt_t[g][ho:ho + 64, qb * 128:qb * 128 + 256],
                                start=True,
                                stop=True,
                            )
                            feed()
                    # e = exp(raw scores) in one ACT pass per head; one DVE
                    # pass applies the 0/1 band mask in place, one reduces
                    # the softmax denominators for both 256-blocks at once.
                    for h in hs:
                        e[h] = attp.tile([128, 2, 256], BF16, tag="e", bufs=4,
                                         name=f"e{h}_{qp}")
                        nc.scalar.activation(e[h][:], ps_s[h][:], EXP)
                    for h in hs:
                        den[h] = statp.tile([128, 2], F32, tag="den", bufs=8,
                                            name=f"den{h}_{qp}")
                        nc.vector.tensor_tensor(out=e[h][:], in0=e[h][:],
                                                in1=mb[:], op=MULT)
                        nc.vector.tensor_reduce(
                            out=den[h][:], in_=e[h][:],
                            axis=mybir.AxisListType.X, op=ADD)
                    for h in hs:
                        rec[h] = statp.tile([128, 2], F32, tag="rec", bufs=8,
                                            name=f"rec{h}_{qp}")
                        nc.vector.reciprocal(rec[h][:], den[h][:])
                    for h in hs:
                        ps_t = psp.tile([128, 512], BF16, tag="t", bufs=2,
                                        name=f"t{h}_{qp}")
                        for i, qb in enumerate(qbs):
                            p = attp.tile([128, 256], BF16, tag="p", bufs=4,
                                          name=f"p{h}_{qb}")
                            nc.vector.tensor_scalar_mul(
                                p[:], e[h][:, i, :], rec[h][:, i:i + 1])
                            nc.tensor.transpose(
                                ps_t[:, i * 256:i * 256 + 128],
                                p[:, 0:128], ident[:])
                            feed()
                            nc.tensor.transpose(
                                ps_t[:, i * 256 + 128:i * 256 + 256],
                                p[:, 128:256], ident[:])
                            feed()
                        nc.vector.tensor_copy(
                            pt_t[h][:, qp * 512 + 128:qp * 512 + 640],
                            ps_t[:])

            def emit_pv(g):
                # P@V with the overlapping 128-col output windows split so
                # adjacent kv-blocks chain-accumulate (region r gets jb=r
                # with start=True, then jb=r+1 with start=False).  start=True
                # clears has_written for the whole PSUM bank, so at most one
                # accumulation group may be open per bank: each head gets its
                # own bank and its region groups open/close strictly in
                # sequence.  h1 uses partitions 64-127 of its bank so the
                # yt copies never shift partitions.
                hs = (2 * g, 2 * g + 1)
                pt_t = pt_all[g]
                ps_y = {h: psp.tile([128, TLOC], F32, tag="y", bufs=2,
                                    name=f"y{g}_{h}")
                        for h in hs}
                for jb in range(NKB):
                    subs = []
                    if jb > 0:
                        subs.append((jb - 1, jb * 256, False, True))
                    if jb < NKB - 1:
                        subs.append((jb, jb * 256 + 128, True, False))
                    for r, c0, st, sp in subs:
                        for h in hs:
                            ho = (h % 2) * 64
                            nc.tensor.matmul(
                                ps_y[h][ho:ho + 64, r * 128:(r + 1) * 128],
                                v_t[jb][:, h * 64:(h + 1) * 64],
                                pt_t[h][:, c0:c0 + 128],
                                start=st,
                                stop=sp,
                                tile_position=(0, ho) if ho else None,
                            )
                            feed()
                yt = ytp.tile([128, TLOC], BF16, name=f"yt{g}", tag="yt",
                              bufs=KT)
                for h in hs:
                    ho = (h % 2) * 64
                    nc.scalar.copy(yt[ho:ho + 64, :], ps_y[h][ho:ho + 64, :])
                yt_t[g] = yt

            # Drains enforce the in-order PE constraint: every feeder matmul
            # a later instruction depends on must already be emitted (V half
            # before pv(4), kt[7]/qt[7] before scores(7)); rationed feeding
            # spreads the rest.
            emit_scores(0)
            for g in range(1, KT):
                if g == 7:
                    drain("k7q7", every=2, open_to="opre")
                emit_scores(g)
                if g == 5:
                    drain("v", every=2, open_to="k7q7")
                emit_pv(g - 1)
            emit_pv(KT - 1)
            drain("opre")
            for n0 in (0, 512):
                for o in (6, 7):
                    ochain[n0][o]()

            # ---- output projection z = y @ Wo^T for tb=1..3 (tb=0 ran
            # through the feeder above)
            for tb in range(1, NQB):
                zt = zp.tile([128, C], F32, name=f"z{tb}", tag="z", bufs=2)
                for n0 in (0, 512):
                    ps = psp.tile([128, 512], F32, tag="proj", bufs=2, name=f"psz{tb}_{n0}")
                    for o in range(KT):
                        nc.tensor.matmul(
                            ps[:],
                            yt_t[o][:, tb * 128:(tb + 1) * 128],
                            wo_t[o][:, n0:n0 + 512],
                            start=(o == 0),
                            stop=(o == KT - 1),
                        )
                    nc.scalar.copy(zt[:, n0:n0 + 512], ps[:])
                    nc.gpsimd.dma_start(
                        out_d[tb * 128:(tb + 1) * 128, n0:n0 + 512],
                        zt[:, n0:n0 + 512])

    nc.compile()
    return nc


def _masks():
    il = np.arange(128)[:, None]
    jl = np.arange(256)[None, :]
    maskr = ((jl > il) & (jl <= il + 128))
    mask0 = (maskr & (jl >= 128))
    mbr = maskr.astype(ml_dtypes.bfloat16)
    mb0 = mask0.astype(ml_dtypes.bfloat16)
    return mb0, mbr


def make_in_maps(x, Wq, Wk, Wv, Wo):
    x = np.asarray(x, dtype=np.float32)
    xt = np.ascontiguousarray(x.reshape(T, C).T.astype(ml_dtypes.bfloat16))
    wqt = np.ascontiguousarray(
        (np.asarray(Wq, np.float32).T * np.float32(1.0 / np.sqrt(DH))
         ).astype(ml_dtypes.bfloat16))
    wkt = np.ascontiguousarray(np.asarray(Wk, np.float32).T.astype(ml_dtypes.bfloat16))
    wvt = np.ascontiguousarray(np.asarray(Wv, np.float32).T.astype(ml_dtypes.bfloat16))
    wot = np.ascontiguousarray(np.asarray(Wo, np.float32).T.astype(ml_dtypes.bfloat16))
    mask0, maskr = _masks()

    in_maps = []
    for c in range(NCORES):
        t0 = c * TLOC
        xs = np.zeros((C, KV), dtype=ml_dtypes.bfloat16)
        lo = t0 - HALO
        src_lo = max(lo, 0)
        xs[:, src_lo - lo:] = xt[:, src_lo:t0 + TLOC]
        in_maps.append(
            {
                "xt": xs,
                "wqt": wqt,
                "wkt": wkt,
                "wvt": wvt,
                "wot": wot,
                "mask0": mask0 if c == 0 else maskr,
                "maskr": maskr,
            }
        )
    return in_maps


def get_nc():
    if "nc" not in _cached:
        _cached["nc"] = build_nc()
    return _cached["nc"]


def kernel(x, Wq, Wk, Wv, Wo):
    in_maps = make_in_maps(x, Wq, Wk, Wv, Wo)
    res = run_bass_kernel_spmd(get_nc(), in_maps, list(range(NCORES)))
    out = np.concatenate([res.results[c]["out"] for c in range(NCORES)], axis=0)
    return out.reshape(1, T, C)


if __name__ == "__main__":
    rng = np.random.default_rng(0)
    ins = {
        "x": rng.standard_normal((1, T, C), dtype=np.float32),
        "Wq": rng.standard_normal((C, C), dtype=np.float32) * 0.02,
        "Wk": rng.standard_normal((C, C), dtype=np.float32) * 0.02,
        "Wv": rng.standard_normal((C, C), dtype=np.float32) * 0.02,
        "Wo": rng.standard_normal((C, C), dtype=np.float32) * 0.02,
    }
    out = kernel(**ins)
    print(out.shape, out.dtype, np.abs(out).mean())


# revision 27
# speedup vs baseline: 1.1865x; 1.1865x over previous
"""Banded causal self-attention (B=1, T=4096, C=1024, H=16, Dh=64, band=128)
on 8 Trainium2 NeuronCores, sequence-parallel (512 queries/core + 128-row halo).

v2: software-pipelined issue order.  The v1 kernel ran all projections, then
all attention, then the output projection; the attention phase's small
matmuls (128-256 free cols, fresh LDWEIGHTS each) left the PE duty cycle low
enough that the HAM clock gate re-throttled to K=4/8 (1.2 GHz) for ~65us.
v2 interleaves the second half of the V projection (dense 512-col matmuls)
into the attention instruction stream at matmul granularity, keeping the PE
busy window saturated so HAM stays at 8/8, and moves P^T copies to the idle
GPSIMD engine, DMA issue off the hot sync queue, and q/k tiles to bf16
(FWL-eligible stationary operands).

Layout strategy (host pre-transposes, so zero on-chip weight transposes):
  - feeds x^T slice (C, 640) per core; Wq^T (scaled by 1/sqrt(Dh)), Wk^T,
    Wv^T, Wo^T as (C, C) contraction-major arrays.
  - q^T/k^T computed as (o, t) tiles; v as (t, o); attention scores banded
    (each 128-query block sees exactly 2 key blocks); softmax along free dim
    without max-subtraction (scores are O(1) by construction); P transposed
    via PE; y^T accumulated per head; output projection back to (t, u).
"""

import os
import sys
from collections import deque

import ml_dtypes
import numpy as np

sys.path.insert(0, "/opt/trn_rl_repo")

import concourse.bass as bass  # noqa: F401
import concourse.mybir as mybir
import concourse.tile as tile
from concourse import bacc
from concourse.bass_utils import run_bass_kernel_spmd
from concourse.masks import make_identity

T, C, H, DH = 4096, 1024, 16, 64
BAND = 128
NCORES = 8
TLOC = T // NCORES          # 512 queries per core
HALO = BAND                 # 128
KV = TLOC + HALO            # 640 kv rows per core
NQB = TLOC // 128           # 4 query blocks
NKB = KV // 128             # 5 kv blocks
KT = C // 128               # 8 contraction tiles
F32 = mybir.dt.float32
BF16 = mybir.dt.bfloat16
MULT = mybir.AluOpType.mult
ADD = mybir.AluOpType.add
EXP = mybir.ActivationFunctionType.Exp

_cached = {}


def build_nc():
    nc = bacc.Bacc(
        "TRN2",
        target_bir_lowering=False,
        debug=False,
        num_devices=NCORES,
    )

    xt_d = nc.dram_tensor("xt", [C, KV], BF16, kind="ExternalInput")
    wqt_d = nc.dram_tensor("wqt", [C, C], BF16, kind="ExternalInput")
    wkt_d = nc.dram_tensor("wkt", [C, C], BF16, kind="ExternalInput")
    wvt_d = nc.dram_tensor("wvt", [C, C], BF16, kind="ExternalInput")
    wot_d = nc.dram_tensor("wot", [C, C], BF16, kind="ExternalInput")
    m0_d = nc.dram_tensor("mask0", [128, 256], BF16, kind="ExternalInput")
    mr_d = nc.dram_tensor("maskr", [128, 256], BF16, kind="ExternalInput")
    out_d = nc.dram_tensor("out", [TLOC, C], F32, kind="ExternalOutput")

    with tile.TileContext(nc) as tc:
        with (
            tc.tile_pool(name="const", bufs=1) as constp,
            tc.tile_pool(name="xt", bufs=KT) as xtp,
            tc.tile_pool(name="w", bufs=32) as wp,
            tc.tile_pool(name="qt", bufs=KT) as qtp,
            tc.tile_pool(name="kt", bufs=KT) as ktp,
            tc.tile_pool(name="v", bufs=NKB) as vp,
            tc.tile_pool(name="yt", bufs=KT) as ytp,
            tc.tile_pool(name="att", bufs=12) as attp,
            tc.tile_pool(name="pt", bufs=4) as ptp,
            tc.tile_pool(name="stat", bufs=8) as statp,
            tc.tile_pool(name="z", bufs=2) as zp,
            tc.tile_pool(name="psum", bufs=1, space="PSUM") as psp,
        ):
            # constants
            ident = constp.tile([128, 128], BF16, name="ident")
            make_identity(nc, ident[:])
            # HAM warm-up: junk matmuls that run while the first DMAs land,
            # flipping the PE clock gate to 8/8 before real work begins
            junk = constp.tile([128, 512], BF16, name="junk")
            nc.vector.memset(junk[:], 0.0)
            ps_w = psp.tile([128, 512], F32, tag="proj", bufs=2, name="warm")
            for _ in range(10):
                nc.tensor.matmul(ps_w[:], junk[:, 0:128], junk[:], start=True,
                                 stop=True)

            # ---- DMA staging, in need-order.  Issue serialization (~0.65us
            # per dma_start) staggers the streams so the first-needed tiles
            # get the HBM bandwidth first: sync issues x^T then wv/wo/masks;
            # the scalar queue issues wq/wk in parallel.
            xt_t = []
            for a in range(KT):
                xt = xtp.tile([128, KV], BF16, name=f"xt{a}", tag="xt", bufs=KT)
                nc.sync.dma_start(xt[:], xt_d[a * 128:(a + 1) * 128, :])
                xt_t.append(xt)

            def load_w(dram, base, k, eng):
                w = wp.tile([128, C], BF16, name=f"{base}{k}", tag="w", bufs=32)
                eng.dma_start(w[:], dram[k * 128:(k + 1) * 128, :])
                return w

            wq_t = [load_w(wqt_d, "wq", k, nc.scalar) for k in range(KT)]
            wk_t = [load_w(wkt_d, "wk", k, nc.scalar) for k in range(KT)]
            wv_t = [load_w(wvt_d, "wv", k, nc.sync) for k in range(KT)]
            wo_t = [load_w(wot_d, "wo", k, nc.sync) for k in range(KT)]

            mb01 = constp.tile([128, 2, 256], BF16, name="mb01")
            mbr2 = constp.tile([128, 2, 256], BF16, name="mbr2")
            nc.sync.dma_start(mb01[:, 0, :], m0_d[:])
            nc.sync.dma_start(mb01[:, 1, :], mr_d[:])
            nc.sync.dma_start(mbr2[:, 0, :], mr_d[:])
            nc.sync.dma_start(mbr2[:, 1, :], mr_d[:])

            qt_t = [None] * KT
            kt_t = [None] * KT
            v_t = []

            # Breadth-first (k-outer) projection passes for Q and K, o=0..6:
            # seven output blocks accumulate in seven PSUM banks at once, so
            # the first matmuls issue as soon as (x^T tile k, W tile k) land
            # instead of waiting for the whole weight matrix.  The attention
            # PSUM tags (proj/s/t/y) are idle this early, so their statically
            # reserved banks are borrowed.  The last two k-steps run per-o so
            # each o closes (and copies out) in ascending order.
            QTAGS = ["proj", "proj", "s", "s", "t", "t", "y"]
            NO7 = KT - 1

            def kouter_pass(mk_mm, mk_fin, tag_name):
                ps = [psp.tile([128, 512], F32, tag=QTAGS[o], bufs=2,
                               name=f"{tag_name}{o}") for o in range(NO7)]
                for k in range(KT - 2):
                    for o in range(NO7):
                        mk_mm(ps[o], o, k, k == 0, False)
                for o in range(NO7):
                    mk_mm(ps[o], o, KT - 2, False, False)
                    mk_mm(ps[o], o, KT - 1, False, True)
                    mk_fin(ps[o], o)

            def q_mm(ps, o, k, st, sp):
                nc.tensor.matmul(
                    ps[:],
                    wq_t[k][:, o * 128:(o + 1) * 128],
                    xt_t[k][:, HALO:],
                    start=st,
                    stop=sp,
                )

            def q_fin(ps, o):
                qt = qtp.tile([128, TLOC], BF16, name=f"qt{o}", tag="qt",
                              bufs=KT)
                nc.scalar.copy(qt[:], ps[:])
                qt_t[o] = qt

            def k_mm(n0, nw):
                def f(ps, o, k, st, sp):
                    nc.tensor.matmul(
                        ps[:, :nw],
                        wk_t[k][:, o * 128:(o + 1) * 128],
                        xt_t[k][:, n0:n0 + nw],
                        start=st,
                        stop=sp,
                    )
                return f

            def k_fin(n0, nw):
                def f(ps, o):
                    if kt_t[o] is None:
                        kt_t[o] = ktp.tile([128, KV], BF16, name=f"kt{o}",
                                           tag="kt", bufs=KT)
                    nc.scalar.copy(kt_t[o][:, n0:n0 + nw], ps[:, :nw])
                return f

            kouter_pass(q_mm, q_fin, "psq")
            kouter_pass(k_mm(0, 384), k_fin(0, 384), "pska")
            kouter_pass(k_mm(384, 256), k_fin(384, 256), "pskb")

            # ---- v projection, first half (output cols 0:512 = heads 0-7):
            # computed up front so attention pairs g=0..3 can run; the second
            # half is emitted lazily through the dense-matmul feeder below,
            # interleaved into the attention phase to keep the PE saturated.
            for tb in range(NKB):
                v = vp.tile([128, C], BF16, name=f"v{tb}", tag="v", bufs=NKB)
                ps = psp.tile([128, 512], F32, tag="proj", bufs=2, name=f"psv{tb}_0")
                for k in range(KT):
                    nc.tensor.matmul(
                        ps[:],
                        xt_t[k][:, tb * 128:(tb + 1) * 128],
                        wv_t[k][:, 0:512],
                        start=(k == 0),
                        stop=(k == KT - 1),
                    )
                nc.scalar.copy(v[:, 0:512], ps[:])
                v_t.append(v)

            # ---- dense-matmul feeder: one closure per PE instruction of
            # deferred dense work (V second half, K o=6,7, Q o=7).  Attention
            # code pulls from this between its own small matmuls so the PE
            # never sees a low-duty window (which would re-throttle HAM), and
            # LDWEIGHTS of the small matmuls hides under the 512-col ones.
            steps = []
            marks = {}

            def _mk_v_chain(tb):
                state = {}

                def step(k):
                    def f():
                        if k == 0:
                            state["ps"] = psp.tile(
                                [128, 512], F32, tag="proj", bufs=2,
                                name=f"psv{tb}_512")
                        nc.tensor.matmul(
                            state["ps"][:],
                            xt_t[k][:, tb * 128:(tb + 1) * 128],
                            wv_t[k][:, 512:1024],
                            start=(k == 0),
                            stop=(k == KT - 1),
                        )
                        if k == KT - 1:
                            nc.scalar.copy(v_t[tb][:, 512:1024], state["ps"][:])
                    return f

                return [step(k) for k in range(KT)]

            def _mk_k_chunk(o, n0, nw):
                state = {}

                def step(k):
                    def f():
                        if kt_t[o] is None:
                            kt_t[o] = ktp.tile([128, KV], BF16, name=f"kt{o}",
                                               tag="kt", bufs=KT)
                        if k == 0:
                            state["ps"] = psp.tile(
                                [128, 512], F32, tag="proj", bufs=2,
                                name=f"psk{o}_{n0}")
                        nc.tensor.matmul(
                            state["ps"][:, :nw],
                            wk_t[k][:, o * 128:(o + 1) * 128],
                            xt_t[k][:, n0:n0 + nw],
                            start=(k == 0),
                            stop=(k == KT - 1),
                        )
                        if k == KT - 1:
                            nc.scalar.copy(kt_t[o][:, n0:n0 + nw],
                                           state["ps"][:, :nw])
                    return f

                return [step(k) for k in range(KT)]

            def _mk_q_chain(o):
                state = {}

                def step(k):
                    def f():
                        if k == 0:
                            state["ps"] = psp.tile(
                                [128, 512], F32, tag="proj", bufs=2,
                                name=f"psq{o}")
                        nc.tensor.matmul(
                            state["ps"][:],
                            wq_t[k][:, o * 128:(o + 1) * 128],
                            xt_t[k][:, HALO:],
                            start=(k == 0),
                            stop=(k == KT - 1),
                        )
                        if k == KT - 1:
                            qt = qtp.tile([128, TLOC], BF16, name=f"qt{o}",
                                          tag="qt", bufs=KT)
                            nc.scalar.copy(qt[:], state["ps"][:])
                            qt_t[o] = qt
                    return f

                return [step(k) for k in range(KT)]

            # O-projection chains for tb=0 (both 512-col halves): the o=0..5
            # matmuls can run during the attention tail (their yt inputs are
            # ready), keeping the PE dense there; o=6,7 + the copy/DMA are
            # emitted after the last PV.
            zt0 = [None]

            def _mk_o_chain(tb, n0):
                state = {}

                def step(o):
                    def f():
                        if o == 0:
                            state["ps"] = psp.tile(
                                [128, 512], F32, tag="proj", bufs=2,
                                name=f"psz{tb}_{n0}")
                        nc.tensor.matmul(
                            state["ps"][:],
                            yt_t[o][:, tb * 128:(tb + 1) * 128],
                            wo_t[o][:, n0:n0 + 512],
                            start=(o == 0),
                            stop=(o == KT - 1),
                        )
                        if o == KT - 1:
                            if zt0[0] is None:
                                zt0[0] = zp.tile([128, C], F32, name=f"z{tb}",
                                                 tag="z", bufs=2)
                            nc.scalar.copy(zt0[0][:, n0:n0 + 512],
                                           state["ps"][:])
                            nc.gpsimd.dma_start(
                                out_d[tb * 128:(tb + 1) * 128, n0:n0 + 512],
                                zt0[0][:, n0:n0 + 512])
                    return f

                return [step(o) for o in range(KT)]

            ochain = {n0: _mk_o_chain(0, n0) for n0 in (0, 512)}

            for tb in range(NKB):
                steps.extend(_mk_v_chain(tb))
            marks["v"] = len(steps)
            for n0, nw in ((0, 384), (384, 256)):
                steps.extend(_mk_k_chunk(KT - 1, n0, nw))
            steps.extend(_mk_q_chain(KT - 1))
            marks["k7q7"] = len(steps)
            for o in range(6):
                steps.append(ochain[0][o])
                steps.append(ochain[512][o])
            marks["opre"] = len(steps)

            # feed() emits dense work in contiguous BURSTS of 8 matmuls
            # (~3.4us of PE-busy, a full HAM activity window): once the clock
            # gate drops to K=4/8 during a sparse stretch, only a sustained
            # dense burst flips it back to 8/8 — thin 1-in-N interleaving
            # keeps the PE "busy" but never re-warms it.  Pops never pass
            # pos["limit"]: a segment only opens once the drain before it has
            # run (its data dependencies are then guaranteed emitted).
            pos = {"i": 0, "tick": 0, "limit": marks["v"]}

            def feed():
                pos["tick"] += 1
                if pos["tick"] % 32 == 0:
                    for _ in range(8):
                        if pos["i"] < pos["limit"]:
                            steps[pos["i"]]()
                            pos["i"] += 1

            def drain(mark, open_to=None):
                end = marks[mark] if mark in marks else len(steps)
                while pos["i"] < end:
                    steps[pos["i"]]()
                    pos["i"] += 1
                pos["limit"] = marks[open_to] if open_to else len(steps)

            # ---- banded attention: head pairs (concurrent PE row-groups)
            # x query-block pairs batched into shared PSUM banks.
            # Pipelined: scores/softmax/transpose for pair g overlap the PV
            # matmuls of pair g-1, with dense V-projection matmuls fed
            # between attention matmuls.
            yt_t = [None] * KT
            pt_all = [None] * KT

            def emit_scores(g):
                hs = (2 * g, 2 * g + 1)
                pt_t = {h: ptp.tile([128, 256 * NKB], BF16,
                                    name=f"pt{h}", tag="pt", bufs=4)
                        for h in hs}
                pt_all[g] = pt_t
                for qp in range(NQB // 2):
                    qbs = (2 * qp, 2 * qp + 1)
                    mb = mb01 if qp == 0 else mbr2
                    ps_s, e, den, rec = {}, {}, {}, {}
                    for h in hs:
                        ho = (h % 2) * 64
                        ps_s[h] = psp.tile([128, 512], F32, tag="s", bufs=2,
                                           name=f"s{h}_{qp}")
                        for i, qb in enumerate(qbs):
                            nc.tensor.matmul(
                                ps_s[h][:, i * 256:(i + 1) * 256],
                                qt_t[g][ho:ho + 64, qb * 128:(qb + 1) * 128],
                                kt_t[g][ho:ho + 64, qb * 128:qb * 128 + 256],
                                start=True,
                                stop=True,
                            )
                            feed()
                    # e = exp(raw scores) in one ACT pass per head; one DVE
                    # pass applies the 0/1 band mask in place, one reduces
                    # the softmax denominators for both 256-blocks at once.
                    for h in hs:
                        e[h] = attp.tile([128, 2, 256], BF16, tag="e", bufs=4,
                                         name=f"e{h}_{qp}")
                        nc.scalar.activation(e[h][:], ps_s[h][:], EXP)
                    for h in hs:
                        den[h] = statp.tile([128, 2], F32, tag="den", bufs=8,
                                            name=f"den{h}_{qp}")
                        nc.vector.tensor_tensor(out=e[h][:], in0=e[h][:],
                                                in1=mb[:], op=MULT)
                        nc.vector.tensor_reduce(
                            out=den[h][:], in_=e[h][:],
                            axis=mybir.AxisListType.X, op=ADD)
                    for h in hs:
                        rec[h] = statp.tile([128, 2], F32, tag="rec", bufs=8,
                                            name=f"rec{h}_{qp}")
                        nc.vector.reciprocal(rec[h][:], den[h][:])
                    for h in hs:
                        ps_t = psp.tile([128, 512], BF16, tag="t", bufs=2,
                                        name=f"t{h}_{qp}")
                        for i, qb in enumerate(qbs):
                            p = attp.tile([128, 256], BF16, tag="p", bufs=4,
                                          name=f"p{h}_{qb}")
                            nc.vector.tensor_scalar_mul(
                                p[:], e[h][:, i, :], rec[h][:, i:i + 1])
                            nc.tensor.transpose(
                                ps_t[:, i * 256:i * 256 + 128],
                                p[:, 0:128], ident[:])
                            feed()
                            nc.tensor.transpose(
                                ps_t[:, i * 256 + 128:i * 256 + 256],
                                p[:, 128:256], ident[:])
                            feed()
                        nc.vector.tensor_copy(
                            pt_t[h][:, qp * 512 + 128:qp * 512 + 640],
                            ps_t[:])

            def emit_pv(g):
                # P@V with the overlapping 128-col output windows split so
                # adjacent kv-blocks chain-accumulate (region r gets jb=r
                # with start=True, then jb=r+1 with start=False).  start=True
                # clears has_written for the whole PSUM bank, so at most one
                # accumulation group may be open per bank: each head gets its
                # own bank and its region groups open/close strictly in
                # sequence.  h1 uses partitions 64-127 of its bank so the
                # yt copies never shift partitions.
                hs = (2 * g, 2 * g + 1)
                pt_t = pt_all[g]
                ps_y = {h: psp.tile([128, TLOC], F32, tag="y", bufs=2,
                                    name=f"y{g}_{h}")
                        for h in hs}
                for jb in range(NKB):
                    subs = []
                    if jb > 0:
                        subs.append((jb - 1, jb * 256, False, True))
                    if jb < NKB - 1:
                        subs.append((jb, jb * 256 + 128, True, False))
                    for r, c0, st, sp in subs:
                        for h in hs:
                            ho = (h % 2) * 64
                            nc.tensor.matmul(
                                ps_y[h][ho:ho + 64, r * 128:(r + 1) * 128],
                                v_t[jb][:, h * 64:(h + 1) * 64],
                                pt_t[h][:, c0:c0 + 128],
                                start=st,
                                stop=sp,
                                tile_position=(0, ho) if ho else None,
                            )
                            feed()
                yt = ytp.tile([128, TLOC], BF16, name=f"yt{g}", tag="yt",
                              bufs=KT)
                for h in hs:
                    ho = (h % 2) * 64
                    nc.scalar.copy(yt[ho:ho + 64, :], ps_y[h][ho:ho + 64, :])
                yt_t[g] = yt

            # Drains enforce the in-order PE constraint: every feeder matmul
            # a later instruction depends on must already be emitted (V half
            # before pv(4), kt[7]/qt[7] before scores(7)); rationed feeding
            # spreads the rest.
            emit_scores(0)
            for g in range(1, KT):
                if g == 7:
                    drain("k7q7", open_to="opre")
                emit_scores(g)
                if g == 5:
                    drain("v", open_to="k7q7")
                emit_pv(g - 1)
            emit_pv(KT - 1)
            drain("opre")
            for n0 in (0, 512):
                for o in (6, 7):
                    ochain[n0][o]()

            # ---- output projection z = y @ Wo^T for tb=1..3 (tb=0 ran
            # through the feeder above)
            for tb in range(1, NQB):
                zt = zp.tile([128, C], F32, name=f"z{tb}", tag="z", bufs=2)
                for n0 in (0, 512):
                    ps = psp.tile([128, 512], F32, tag="proj", bufs=2, name=f"psz{tb}_{n0}")
                    for o in range(KT):
                        nc.tensor.matmul(
                            ps[:],
                            yt_t[o][:, tb * 128:(tb + 1) * 128],
                            wo_t[o][:, n0:n0 + 512],
                            start=(o == 0),
                            stop=(o == KT - 1),
                        )
                    nc.scalar.copy(zt[:, n0:n0 + 512], ps[:])
                    nc.gpsimd.dma_start(
                        out_d[tb * 128:(tb + 1) * 128, n0:n0 + 512],
                        zt[:, n0:n0 + 512])

    nc.compile()
    return nc


def _masks():
    il = np.arange(128)[:, None]
    jl = np.arange(256)[None, :]
    maskr = ((jl > il) & (jl <= il + 128))
    mask0 = (maskr & (jl >= 128))
    mbr = maskr.astype(ml_dtypes.bfloat16)
    mb0 = mask0.astype(ml_dtypes.bfloat16)
    return mb0, mbr


def make_in_maps(x, Wq, Wk, Wv, Wo):
    x = np.asarray(x, dtype=np.float32)
    xt = np.ascontiguousarray(x.reshape(T, C).T.astype(ml_dtypes.bfloat16))
    wqt = np.ascontiguousarray(
        (np.asarray(Wq, np.float32).T * np.float32(1.0 / np.sqrt(DH))
         ).astype(ml_dtypes.bfloat16))
    wkt = np.ascontiguousarray(np.asarray(Wk, np.float32).T.astype(ml_dtypes.bfloat16))
    wvt = np.ascontiguousarray(np.asarray(Wv, np.float32).T.astype(ml_dtypes.bfloat16))
    wot = np.ascontiguousarray(np.asarray(Wo, np.float32).T.astype(ml_dtypes.bfloat16))
    mask0, maskr = _masks()

    in_maps = []
    for c in range(NCORES):
        t0 = c * TLOC
        xs = np.zeros((C, KV), dtype=ml_dtypes.bfloat16)
        lo = t0 - HALO
        src_lo = max(lo, 0)
        xs[:, src_lo - lo:] = xt[:, src_lo:t0 + TLOC]
        in_maps.append(
            {
                "xt": xs,
                "wqt": wqt,
                "wkt": wkt,
                "wvt": wvt,
                "wot": wot,
                "mask0": mask0 if c == 0 else maskr,
                "maskr": maskr,
            }
        )
    return in_maps


def get_nc():
    if "nc" not in _cached:
        _cached["nc"] = build_nc()
    return _cached["nc"]


def kernel(x, Wq, Wk, Wv, Wo):
    in_maps = make_in_maps(x, Wq, Wk, Wv, Wo)
    res = run_bass_kernel_spmd(get_nc(), in_maps, list(range(NCORES)))
    out = np.concatenate([res.results[c]["out"] for c in range(NCORES)], axis=0)
    return out.reshape(1, T, C)


if __name__ == "__main__":
    rng = np.random.default_rng(0)
    ins = {
        "x": rng.standard_normal((1, T, C), dtype=np.float32),
        "Wq": rng.standard_normal((C, C), dtype=np.float32) * 0.02,
        "Wk": rng.standard_normal((C, C), dtype=np.float32) * 0.02,
        "Wv": rng.standard_normal((C, C), dtype=np.float32) * 0.02,
        "Wo": rng.standard_normal((C, C), dtype=np.float32) * 0.02,
    }
    out = kernel(**ins)
    print(out.shape, out.dtype, np.abs(out).mean())
